# revision 24
# baseline (speedup 1.0000x reference)
"""Trainium2 Bass kernel for nn_AttentionBlock (B=1, C=512, T=8, H=W=64).

Math: the reference's attention has seq-len 1 (softmax over a single
element == 1.0), so o == v and Q/K never affect the output:

    out = x + s(px) * (W_eff @ x)(px) + b_eff
    W_eff = w_proj @ w_v * gamma,  w_v = w_qkv[2C:3C]
    b_eff = w_proj @ b_v + b_proj
    s(px) = sqrt(C) / clip(||x[:, px]||, 1e-12)

(The per-pixel RMS scale s commutes through the channel contraction, so
the GEMM runs on raw x and s is applied to the GEMM output.)

Sharding: data-parallel over the fused (b*t)=8 frame axis, one frame per
NeuronCore; weights replicated. Per core the frame is shipped tile-major
([tile, p, chunk, n]: channels on partitions, pixels on the free dim).

Precision plan (rel-l2 budget is 2e-2; this lands ~8e-3):
 - residual identity + final store ride in bf16 (two roundings ~3e-3)
 - the GEMM runs in fp8e4m3 with DoubleRow perf mode (2 k-chunks per
   pass). W_eff is pre-scaled by 2^6 on the host — its entries (~1e-2)
   sit below e4m3's min normal otherwise — and the 2^-6 descale is
   folded into the Sqrt scale of the norm chain for free.
 - sumsq is computed from the fp8 copy of x (error ~2e-3 on s): the
   fp8 tile is a quarter the bytes and is shipped first, so the
   partition-reducing ones-matmuls never wait on the big bf16 tile and
   the warm PE never starves (a >3.4us PE gap would re-throttle the
   HAM clock gate to 1.2GHz).

Per-tile engine budget (8 tiles/core), balanced with on-HW measurements
(GPSIMD tensor ops run ~2.3x slower than DVE's; DVE all-bf16 ops get
2x mode; the psum-reading scale muls must be DVE):
  PE      8 DoubleRow mains + 4 sumsq-ones MMs    ~3.0us
  ACT     square (fp8->bf16) + sqrt               ~2.7us + store issue
  DVE     2 psum-scale muls + 1/4 residual add + reciprocal ~3.5us
  GPSIMD  3/4 residual add                        ~3.1us
Loads stream on the qSP HWDGE ring ([x8_t, x_t] per tile, weights after
x8_0); stores are issued per-tile on the qAct ring (one tile delayed so
a still-waiting store never head-of-line blocks the ACT compute queue)
and drain concurrently with the loads. A burst of throwaway matmuls on
constants bridges the preamble-to-first-data window so the PE's HAM
clock gate is already at 8/8 when the real stream begins.
"""

import numpy as np

import concourse.tile as tile
from concourse import bacc, mybir
from concourse.bass_utils import run_bass_kernel_spmd

C = 512  # channels
T = 8  # frames == cores
PX = 4096  # pixels per frame (64*64)
NT = 512  # pixel-tile (one PSUM bank of fp32)
NTILES = PX // NT  # 8
KC = C // 128  # 4 channel chunks

F32 = mybir.dt.float32
F32R = mybir.dt.float32r
BF16 = mybir.dt.bfloat16
FP8 = mybir.dt.float8e4

WSHIFT = 64.0  # w8 = W_eff.T * 2^6; descale folded into the Sqrt scale

# 1e-24/C: Sqrt((sumsq + 1e-24)/C) reproduces the reference's
# clip(norm, 1e-12) for all non-degenerate inputs. The norm chain
# computes WSHIFT*sqrt(.) so the reciprocal yields s/WSHIFT directly.
_EPS = 1e-24 / C

_BUILD_CACHE: dict = {}


def _build(has_bias: bool):
    """Trace + compile the per-core Tile program. Returns the Bacc."""
    nc = bacc.Bacc("TRN2", target_bir_lowering=False, debug=False, num_devices=T)

    # x/x8/out are tile-major on the host side ([tile, p, a, n]): each
    # pixel tile is one contiguous DRAM block with a single contiguous
    # line per partition. w8 is [p, a, j, m]: one contiguous 2KB row per
    # partition.
    x = nc.dram_tensor("x", [NTILES, 128, KC, NT], BF16, kind="ExternalInput").ap()
    x8 = nc.dram_tensor("x8", [NTILES, 128, KC, NT], FP8, kind="ExternalInput").ap()
    w8 = nc.dram_tensor("w8", [128, KC, KC, 128], FP8, kind="ExternalInput").ap()
    out = nc.dram_tensor("out", [NTILES, 128, KC, NT], BF16, kind="ExternalOutput").ap()
    beff = None
    if has_bias:
        beff = nc.dram_tensor("beff", [128, KC], BF16, kind="ExternalInput").ap()

    with tile.TileContext(nc) as tc:
        with (
            tc.tile_pool(name="const", bufs=1) as const,
            tc.tile_pool(name="xin", bufs=8) as xin,
            tc.tile_pool(name="x8in", bufs=8) as x8in,
            tc.tile_pool(name="sq", bufs=3) as sq,
            tc.tile_pool(name="sca", bufs=4) as sca,
            tc.tile_pool(name="tmp", bufs=3) as tmpp,
            tc.tile_pool(name="acc", bufs=3, space="PSUM") as accp,
            tc.tile_pool(name="stat", bufs=2, space="PSUM") as statp,
        ):
            ones_bf = const.tile([128, 128], F32)
            nc.vector.memset(ones_bf, 1.0)
            ones_b = const.tile([128, 128], BF16)
            nc.vector.tensor_copy(ones_b, ones_bf)
            eps_t = const.tile([128, 1], F32)
            nc.vector.memset(eps_t, _EPS * WSHIFT * WSHIFT)
            w8_sb = const.tile([128, KC, KC, 128], FP8)
            if has_bias:
                beff_sb = const.tile([128, KC], BF16)
                nc.sync.dma_start(out=beff_sb, in_=beff)

            # HAM warmup: the PE sits idle from the end of the framework
            # preamble (~7us) until the first operands land (~10.5us),
            # and the clock gate needs ~3.4us of sustained activity to
            # lift 1.2GHz -> 2.4GHz. Throwaway matmuls on constant tiles
            # bridge exactly that window so the real GEMM stream runs
            # warm from its first instruction.
            scr = const.tile([128, NT], BF16)
            nc.vector.memset(scr, 0.0)
            wacc = statp.tile([128, NT], F32, tag="stat", name="warm")
            for _ in range(8):
                nc.tensor.matmul(wacc, lhsT=ones_b, rhs=scr, start=True, stop=True)

            store_q = []  # (ti, xt) pairs whose store is still to be issued

            for ti in range(NTILES):
                x8t = x8in.tile([128, KC, NT], FP8, tag="x8t")
                nc.sync.dma_start(out=x8t, in_=x8[ti])
                if ti == 0:
                    nc.sync.dma_start(out=w8_sb, in_=w8)
                xt = xin.tile([128, KC, NT], BF16, tag="xt")
                nc.sync.dma_start(out=xt, in_=x[ti])

                # per-pixel sum of squares over channels, from the fp8
                # copy (x8 = quant(x), so x8^2 mis-estimates sumsq by
                # ~0.2% — far inside the error budget — and arrives a
                # full bf16-tile earlier): square on ACT, then four
                # ones[128,128] matmuls that reduce the partitions AND
                # broadcast the result (fp32 psum) to every partition.
                x2 = sq.tile([128, KC, NT], BF16, tag="x2", name="x2")
                nc.scalar.activation(
                    out=x2,
                    in_=x8t,
                    func=mybir.ActivationFunctionType.Square,
                )

                accs = []
                for jj in range(KC // 2):
                    accs.append(accp.tile([128, 2, NT], F32, tag="acc", name="acc"))
                ssb = statp.tile([128, NT], F32, tag="stat", name="ssb")

                # sumsq matmuls go FIRST on the PE queue (except tile 0,
                # where waiting for the first ACT square would delay the
                # very first GEMM): the scale chain (sqrt, recip) then
                # completes under the mains, so each combine fires the
                # moment its psum group stops and the psum ring buffer
                # (reused by tile t+1's second group) frees in time.
                def emit_ones():
                    for a in range(KC):
                        nc.tensor.matmul(
                            ssb,
                            lhsT=ones_b,
                            rhs=x2[:, a, :],
                            start=(a == 0),
                            stop=(a == KC - 1),
                        )

                def emit_mains(jjs):
                    # DoubleRow: each pass consumes a pair of ci-chunks
                    # (lhsT [128, 2, 128], rhs [128, 2, 512]), so a psum
                    # group accumulates in 2 passes instead of 4.
                    for jj in jjs:
                        for q in range(2):
                            for p in range(KC // 2):
                                nc.tensor.matmul(
                                    accs[jj][:, q, :],
                                    lhsT=w8_sb[:, 2 * p : 2 * p + 2, jj * 2 + q, :],
                                    rhs=x8t[:, 2 * p : 2 * p + 2, :],
                                    start=(p == 0),
                                    stop=(p == KC // 2 - 1),
                                    perf_mode=mybir.MatmulPerfMode.DoubleRow,
                                )

                def emit_mains_half(h):
                    # last tile only: column-halved psum groups, so the
                    # first half's combine+store overlaps the second
                    # half's matmuls instead of serializing after them.
                    lo, hi = h * (NT // 2), (h + 1) * (NT // 2)
                    for jj in range(KC // 2):
                        for q in range(2):
                            for p in range(KC // 2):
                                nc.tensor.matmul(
                                    accs[jj][:, q, lo:hi],
                                    lhsT=w8_sb[:, 2 * p : 2 * p + 2, jj * 2 + q, :],
                                    rhs=x8t[:, 2 * p : 2 * p + 2, lo:hi],
                                    start=(p == 0),
                                    stop=(p == KC // 2 - 1),
                                    perf_mode=mybir.MatmulPerfMode.DoubleRow,
                                )

                if ti == 0:
                    # tile 0: sumsq slots between the two psum groups —
                    # its square input is ready by then (waiting for it
                    # up front would delay the very first GEMM), and the
                    # scale chain starts ~1.3us earlier than fully-after,
                    # which shrinks tile 1's second-group psum wait.
                    emit_mains([0])
                    emit_ones()
                    emit_mains([1])
                elif ti == NTILES - 1:
                    emit_ones()
                    emit_mains_half(0)
                    emit_mains_half(1)
                else:
                    emit_ones()
                    emit_mains(list(range(KC // 2)))

                # WSHIFT/sqrt-chain: stb = WSHIFT*sqrt(sumsq/C + eps),
                # so recip gives s/WSHIFT and the psum descale is free.
                stb = sca.tile([128, NT], F32R, tag="stb", name="stb")
                nc.scalar.activation(
                    out=stb,
                    in_=ssb,
                    func=mybir.ActivationFunctionType.Sqrt,
                    scale=WSHIFT * WSHIFT / C,
                    bias=eps_t,
                )
                sb_s = sca.tile([128, NT], F32, tag="sb_s", name="sb_s")
                nc.vector.reciprocal_approx_fast(out=sb_s, in_=stb.bitcast(F32))

                # combine: out = x + (s/WSHIFT)*acc (+beff). The
                # psum-reading muls must be DVE; the residual add is
                # split 3:1 GPSIMD:DVE to keep DVE at the PE's pace.
                tm = tmpp.tile([128, KC, NT], BF16, tag="tm", name="tm")
                if ti == NTILES - 1:
                    HN = NT // 2
                    for h in range(2):
                        lo, hi = h * HN, (h + 1) * HN
                        sbw_h = sb_s[:, lo:hi].unsqueeze(1).broadcast_to([128, 2, HN])
                        nc.vector.tensor_mul(tm[:, 0:2, lo:hi], accs[0][:, :, lo:hi], sbw_h)
                        nc.vector.tensor_mul(tm[:, 2:4, lo:hi], accs[1][:, :, lo:hi], sbw_h)
                        nc.vector.tensor_add(
                            xt[:, 0:2, lo:hi], tm[:, 0:2, lo:hi], xt[:, 0:2, lo:hi]
                        )
                        nc.vector.tensor_add(
                            xt[:, 2:4, lo:hi], tm[:, 2:4, lo:hi], xt[:, 2:4, lo:hi]
                        )
                else:
                    sb_w = sb_s.unsqueeze(1).broadcast_to([128, 2, NT])
                    nc.vector.tensor_mul(tm[:, 0:2, :], accs[0], sb_w)
                    nc.vector.tensor_mul(tm[:, 2:4, :], accs[1], sb_w)
                if has_bias:
                    for j in range(KC):
                        nc.vector.tensor_add(
                            tm[:, j, :],
                            tm[:, j, :],
                            beff_sb[:, j : j + 1].broadcast_to([128, NT]),
                        )
                if ti < NTILES - 1:
                    nc.gpsimd.tensor_add(xt[:, 0:2, :], tm[:, 0:2, :], xt[:, 0:2, :])
                    nc.vector.tensor_add(xt[:, 2:4, :], tm[:, 2:4, :], xt[:, 2:4, :])
                else:
                    # the last tile's combine is on the critical path to
                    # the final store: all on DVE (2x-mode bf16), in
                    # column halves matching the halved psum groups.
                    pass

                store_q.append((ti, xt))

            # stores ride the idle qSP ring, emitted after every load so
            # a waiting store can never block a load issue; the SP engine
            # just paces them behind each tile's combine. The last tile
            # stores in halves, each as soon as its add lands.
            for si, sxt in store_q[:-1]:
                nc.sync.dma_start(out=out[si], in_=sxt)
            (si, sxt) = store_q[-1]
            HN = NT // 2
            nc.sync.dma_start(out=out[si][:, :, 0:HN], in_=sxt[:, :, 0:HN])
            nc.sync.dma_start(out=out[si][:, :, HN:NT], in_=sxt[:, :, HN:NT])

    nc.compile()
    return nc


def _get_nc(has_bias: bool):
    key = has_bias
    if key not in _BUILD_CACHE:
        _BUILD_CACHE[key] = _build(has_bias)
    return _BUILD_CACHE[key]


def _prep(x, gamma, w_qkv, b_qkv, w_proj, b_proj):
    """Host-side shard + weight fold. Returns (in_maps, has_bias)."""
    bf16 = mybir.dt.np(BF16)
    fp8 = mybir.dt.np(FP8)
    x = np.asarray(x, dtype=np.float32)
    gamma = np.asarray(gamma, dtype=np.float32)
    w_qkv = np.asarray(w_qkv, dtype=np.float32)
    b_qkv = np.asarray(b_qkv, dtype=np.float32)
    w_proj = np.asarray(w_proj, dtype=np.float32)
    b_proj = np.asarray(b_proj, dtype=np.float32)

    w_v = w_qkv[2 * C : 3 * C, :]  # [cv, ci]
    b_v = b_qkv[2 * C : 3 * C]
    w_eff = (w_proj @ w_v) * gamma[None, :]  # [co, ci]
    # lhsT layout [p, a, j, m]: lhsT[a*128+p -> ci, j*128+m -> co].
    # Pre-scaled by WSHIFT so the ~1e-2 entries clear e4m3's min normal.
    w8 = np.ascontiguousarray(
        (w_eff.T * WSHIFT).reshape(KC, 128, KC, 128).transpose(1, 0, 2, 3).astype(fp8)
    )
    b_eff = (w_proj @ b_v + b_proj).astype(np.float32)
    has_bias = bool(np.any(b_eff != 0.0))

    xb = x.astype(bf16)
    x8 = x.astype(fp8)
    in_maps = []
    for t in range(T):
        shard = xb[0, :, t, :, :].reshape(C, PX)
        xh = np.ascontiguousarray(
            shard.reshape(KC, 128, NTILES, NT).transpose(2, 1, 0, 3)
        )
        shard8 = x8[0, :, t, :, :].reshape(C, PX)
        x8h = np.ascontiguousarray(
            shard8.reshape(KC, 128, NTILES, NT).transpose(2, 1, 0, 3)
        )
        m = {
            "x": xh,
            "x8": x8h,
            "w8": w8,
        }
        if has_bias:
            m["beff"] = np.ascontiguousarray(b_eff.reshape(KC, 128).T.astype(bf16))
        in_maps.append(m)
    return in_maps, has_bias


def _run(inputs: dict, **run_kwargs):
    in_maps, has_bias = _prep(**inputs)
    nc = _get_nc(has_bias)
    res = run_bass_kernel_spmd(nc, in_maps, core_ids=list(range(T)), **run_kwargs)
    b, c, t, h, w = 1, C, T, 64, 64
    out = np.empty((b, c, t, h, w), dtype=np.float32)
    for i in range(T):
        oh = res.results[i]["out"].astype(np.float32)  # [NTILES, 128, KC, NT]
        shard = oh.transpose(2, 1, 0, 3).reshape(c, PX)
        out[0, :, i, :, :] = shard.reshape(c, h, w)
    return out, res


def kernel(**inputs) -> np.ndarray:
    out, _ = _run(inputs)
    return out


# revision 26
# speedup vs baseline: 1.0013x; 1.0013x over previous
"""Trainium2 Bass kernel for nn_AttentionBlock (B=1, C=512, T=8, H=W=64).

Math: the reference's attention has seq-len 1 (softmax over a single
element == 1.0), so o == v and Q/K never affect the output:

    out = x + s(px) * (W_eff @ x)(px) + b_eff
    W_eff = w_proj @ w_v * gamma,  w_v = w_qkv[2C:3C]
    b_eff = w_proj @ b_v + b_proj
    s(px) = sqrt(C) / clip(||x[:, px]||, 1e-12)

(The per-pixel RMS scale s commutes through the channel contraction, so
the GEMM runs on raw x and s is applied to the GEMM output.)

Sharding: data-parallel over the fused (b*t)=8 frame axis, one frame per
NeuronCore; weights replicated. Per core the frame is shipped tile-major
([tile, p, chunk, n]: channels on partitions, pixels on the free dim).

Precision plan (rel-l2 budget is 2e-2; this lands ~8e-3):
 - residual identity + final store ride in bf16 (two roundings ~3e-3)
 - the GEMM runs in fp8e4m3 with DoubleRow perf mode (2 k-chunks per
   pass). W_eff is pre-scaled by 2^6 on the host — its entries (~1e-2)
   sit below e4m3's min normal otherwise — and the 2^-6 descale is
   folded into the Sqrt scale of the norm chain for free.
 - sumsq is computed from the fp8 copy of x (error ~2e-3 on s): the
   fp8 tile is a quarter the bytes and is shipped first, so the
   partition-reducing ones-matmuls never wait on the big bf16 tile and
   the warm PE never starves (a >3.4us PE gap would re-throttle the
   HAM clock gate to 1.2GHz).

Per-tile engine budget (8 tiles/core), balanced with on-HW measurements
(GPSIMD tensor ops run ~2.3x slower than DVE's; DVE all-bf16 ops get
2x mode; the psum-reading scale muls must be DVE):
  PE      8 DoubleRow mains + 4 sumsq-ones MMs    ~3.0us
  ACT     square (fp8->bf16) + sqrt               ~2.7us + store issue
  DVE     2 psum-scale muls + 1/4 residual add + reciprocal ~3.5us
  GPSIMD  3/4 residual add                        ~3.1us
Loads stream on the qSP HWDGE ring ([x8_t, x_t] per tile, weights after
x8_0); stores are issued per-tile on the qAct ring (one tile delayed so
a still-waiting store never head-of-line blocks the ACT compute queue)
and drain concurrently with the loads. A burst of throwaway matmuls on
constants bridges the preamble-to-first-data window so the PE's HAM
clock gate is already at 8/8 when the real stream begins.
"""

import numpy as np

import concourse.tile as tile
from concourse import bacc, mybir
from concourse.bass_utils import run_bass_kernel_spmd

C = 512  # channels
T = 8  # frames == cores
PX = 4096  # pixels per frame (64*64)
NT = 512  # pixel-tile (one PSUM bank of fp32)
NTILES = PX // NT  # 8
KC = C // 128  # 4 channel chunks

F32 = mybir.dt.float32
F32R = mybir.dt.float32r
BF16 = mybir.dt.bfloat16
FP8 = mybir.dt.float8e4

WSHIFT = 64.0  # w8 = W_eff.T * 2^6; descale folded into the Sqrt scale

# 1e-24/C: Sqrt((sumsq + 1e-24)/C) reproduces the reference's
# clip(norm, 1e-12) for all non-degenerate inputs. The norm chain
# computes WSHIFT*sqrt(.) so the reciprocal yields s/WSHIFT directly.
_EPS = 1e-24 / C

_BUILD_CACHE: dict = {}


def _build(has_bias: bool):
    """Trace + compile the per-core Tile program. Returns the Bacc."""
    nc = bacc.Bacc("TRN2", target_bir_lowering=False, debug=False, num_devices=T)

    # x/x8/out are tile-major on the host side ([tile, p, a, n]): each
    # pixel tile is one contiguous DRAM block with a single contiguous
    # line per partition. w8 is [p, a, j, m]: one contiguous 2KB row per
    # partition.
    x = nc.dram_tensor("x", [NTILES, 128, KC, NT], BF16, kind="ExternalInput").ap()
    x8 = nc.dram_tensor("x8", [NTILES, 128, KC, NT], FP8, kind="ExternalInput").ap()
    w8 = nc.dram_tensor("w8", [128, KC, KC, 128], FP8, kind="ExternalInput").ap()
    out = nc.dram_tensor("out", [NTILES, 128, KC, NT], BF16, kind="ExternalOutput").ap()
    beff = None
    if has_bias:
        beff = nc.dram_tensor("beff", [128, KC], BF16, kind="ExternalInput").ap()

    with tile.TileContext(nc) as tc:
        with (
            tc.tile_pool(name="const", bufs=1) as const,
            tc.tile_pool(name="xin", bufs=8) as xin,
            tc.tile_pool(name="x8in", bufs=8) as x8in,
            tc.tile_pool(name="sq", bufs=3) as sq,
            tc.tile_pool(name="sca", bufs=4) as sca,
            tc.tile_pool(name="tmp", bufs=3) as tmpp,
            tc.tile_pool(name="acc", bufs=3, space="PSUM") as accp,
            tc.tile_pool(name="stat", bufs=2, space="PSUM") as statp,
        ):
            ones_bf = const.tile([128, 128], F32)
            nc.vector.memset(ones_bf, 1.0)
            ones_b = const.tile([128, 128], BF16)
            nc.vector.tensor_copy(ones_b, ones_bf)
            eps_t = const.tile([128, 1], F32)
            nc.vector.memset(eps_t, _EPS * WSHIFT * WSHIFT)
            w8_sb = const.tile([128, KC, KC, 128], FP8)
            if has_bias:
                beff_sb = const.tile([128, KC], BF16)
                nc.sync.dma_start(out=beff_sb, in_=beff)

            # HAM warmup: the PE sits idle from the end of the framework
            # preamble (~7us) until the first operands land (~10.5us),
            # and the clock gate needs ~3.4us of sustained activity to
            # lift 1.2GHz -> 2.4GHz. Throwaway matmuls on constant tiles
            # bridge exactly that window so the real GEMM stream runs
            # warm from its first instruction.
            scr = const.tile([128, NT], BF16)
            nc.vector.memset(scr, 0.0)
            wacc = statp.tile([128, NT], F32, tag="stat", name="warm")
            for _ in range(8):
                nc.tensor.matmul(wacc, lhsT=ones_b, rhs=scr, start=True, stop=True)

            store_q = []  # (ti, xt) pairs whose store is still to be issued
            pending_add = None  # previous tile's (xt, tm) residual add

            for ti in range(NTILES):
                x8t = x8in.tile([128, KC, NT], FP8, tag="x8t")
                nc.sync.dma_start(out=x8t, in_=x8[ti])
                if ti == 0:
                    nc.sync.dma_start(out=w8_sb, in_=w8)
                xt = xin.tile([128, KC, NT], BF16, tag="xt")
                nc.sync.dma_start(out=xt, in_=x[ti])

                # per-pixel sum of squares over channels, from the fp8
                # copy (x8 = quant(x), so x8^2 mis-estimates sumsq by
                # ~0.2% — far inside the error budget — and arrives a
                # full bf16-tile earlier): square on ACT, then four
                # ones[128,128] matmuls that reduce the partitions AND
                # broadcast the result (fp32 psum) to every partition.
                x2 = sq.tile([128, KC, NT], BF16, tag="x2", name="x2")
                nc.scalar.activation(
                    out=x2,
                    in_=x8t,
                    func=mybir.ActivationFunctionType.Square,
                )

                accs = []
                for jj in range(KC // 2):
                    accs.append(accp.tile([128, 2, NT], F32, tag="acc", name="acc"))
                ssb = statp.tile([128, NT], F32, tag="stat", name="ssb")

                # sumsq matmuls go FIRST on the PE queue (except tile 0,
                # where waiting for the first ACT square would delay the
                # very first GEMM): the scale chain (sqrt, recip) then
                # completes under the mains, so each combine fires the
                # moment its psum group stops and the psum ring buffer
                # (reused by tile t+1's second group) frees in time.
                def emit_ones():
                    for a in range(KC):
                        nc.tensor.matmul(
                            ssb,
                            lhsT=ones_b,
                            rhs=x2[:, a, :],
                            start=(a == 0),
                            stop=(a == KC - 1),
                        )

                def emit_mains(jjs):
                    # DoubleRow: each pass consumes a pair of ci-chunks
                    # (lhsT [128, 2, 128], rhs [128, 2, 512]), so a psum
                    # group accumulates in 2 passes instead of 4.
                    for jj in jjs:
                        for q in range(2):
                            for p in range(KC // 2):
                                nc.tensor.matmul(
                                    accs[jj][:, q, :],
                                    lhsT=w8_sb[:, 2 * p : 2 * p + 2, jj * 2 + q, :],
                                    rhs=x8t[:, 2 * p : 2 * p + 2, :],
                                    start=(p == 0),
                                    stop=(p == KC // 2 - 1),
                                    perf_mode=mybir.MatmulPerfMode.DoubleRow,
                                )

                if ti == 0:
                    # tile 0: sumsq slots between the two psum groups —
                    # its square input is ready by then (waiting for it
                    # up front would delay the very first GEMM), and the
                    # scale chain starts ~1.3us earlier than fully-after,
                    # which shrinks tile 1's second-group psum wait.
                    emit_mains([0])
                    emit_ones()
                    emit_mains([1])
                else:
                    emit_ones()
                    emit_mains(list(range(KC // 2)))

                # WSHIFT/sqrt-chain: stb = WSHIFT*sqrt(sumsq/C + eps),
                # so recip gives s/WSHIFT and the psum descale is free.
                stb = sca.tile([128, NT], F32R, tag="stb", name="stb")
                nc.scalar.activation(
                    out=stb,
                    in_=ssb,
                    func=mybir.ActivationFunctionType.Sqrt,
                    scale=WSHIFT * WSHIFT / C,
                    bias=eps_t,
                )
                sb_s = sca.tile([128, NT], F32, tag="sb_s", name="sb_s")
                nc.vector.reciprocal_approx_fast(out=sb_s, in_=stb.bitcast(F32))

                # combine: out = x + (s/WSHIFT)*acc (+beff). The
                # psum-reading muls must be DVE; the residual add is
                # split 3:1 GPSIMD:DVE to keep DVE at the PE's pace.
                sb_w = sb_s.unsqueeze(1).broadcast_to([128, 2, NT])
                tm = tmpp.tile([128, KC, NT], BF16, tag="tm", name="tm")
                nc.vector.tensor_mul(tm[:, 0:2, :], accs[0], sb_w)
                # the previous tile's residual adds slot in HERE on the
                # DVE/GPSIMD queues — after this tile's first psum mul.
                # That mul releases the psum ring buffer that tile t+1's
                # second GEMM group waits on, so it must never queue
                # behind an add; the adds fill the DVE idle window
                # before the second mul instead.
                if pending_add is not None:
                    pxt, ptm = pending_add
                    nc.gpsimd.tensor_add(pxt[:, 0:2, :], ptm[:, 0:2, :], pxt[:, 0:2, :])
                    nc.vector.tensor_add(pxt[:, 2:4, :], ptm[:, 2:4, :], pxt[:, 2:4, :])
                    pending_add = None
                nc.vector.tensor_mul(tm[:, 2:4, :], accs[1], sb_w)
                if has_bias:
                    for j in range(KC):
                        nc.vector.tensor_add(
                            tm[:, j, :],
                            tm[:, j, :],
                            beff_sb[:, j : j + 1].broadcast_to([128, NT]),
                        )
                if ti < NTILES - 1:
                    pending_add = (xt, tm)
                else:
                    # the last tile's combine is on the critical path to
                    # the final store: run both halves on DVE (2x-mode
                    # bf16, ~3x faster than GPSIMD) and store each half
                    # as soon as its add lands.
                    nc.vector.tensor_add(xt[:, 0:2, :], tm[:, 0:2, :], xt[:, 0:2, :])
                    nc.vector.tensor_add(xt[:, 2:4, :], tm[:, 2:4, :], xt[:, 2:4, :])

                store_q.append((ti, xt))

            # stores ride the idle qSP ring, emitted after every load so
            # a waiting store can never block a load issue; the SP engine
            # just paces them behind each tile's combine. The last tile
            # stores in halves, each as soon as its add lands.
            for si, sxt in store_q[:-1]:
                nc.sync.dma_start(out=out[si], in_=sxt)
            (si, sxt) = store_q[-1]
            nc.sync.dma_start(out=out[si][:, 0:2, :], in_=sxt[:, 0:2, :])
            nc.sync.dma_start(out=out[si][:, 2:4, :], in_=sxt[:, 2:4, :])

    nc.compile()
    return nc


def _get_nc(has_bias: bool):
    key = has_bias
    if key not in _BUILD_CACHE:
        _BUILD_CACHE[key] = _build(has_bias)
    return _BUILD_CACHE[key]


def _prep(x, gamma, w_qkv, b_qkv, w_proj, b_proj):
    """Host-side shard + weight fold. Returns (in_maps, has_bias)."""
    bf16 = mybir.dt.np(BF16)
    fp8 = mybir.dt.np(FP8)
    x = np.asarray(x, dtype=np.float32)
    gamma = np.asarray(gamma, dtype=np.float32)
    w_qkv = np.asarray(w_qkv, dtype=np.float32)
    b_qkv = np.asarray(b_qkv, dtype=np.float32)
    w_proj = np.asarray(w_proj, dtype=np.float32)
    b_proj = np.asarray(b_proj, dtype=np.float32)

    w_v = w_qkv[2 * C : 3 * C, :]  # [cv, ci]
    b_v = b_qkv[2 * C : 3 * C]
    w_eff = (w_proj @ w_v) * gamma[None, :]  # [co, ci]
    # lhsT layout [p, a, j, m]: lhsT[a*128+p -> ci, j*128+m -> co].
    # Pre-scaled by WSHIFT so the ~1e-2 entries clear e4m3's min normal.
    w8 = np.ascontiguousarray(
        (w_eff.T * WSHIFT).reshape(KC, 128, KC, 128).transpose(1, 0, 2, 3).astype(fp8)
    )
    b_eff = (w_proj @ b_v + b_proj).astype(np.float32)
    has_bias = bool(np.any(b_eff != 0.0))

    xb = x.astype(bf16)
    x8 = x.astype(fp8)
    in_maps = []
    for t in range(T):
        shard = xb[0, :, t, :, :].reshape(C, PX)
        xh = np.ascontiguousarray(
            shard.reshape(KC, 128, NTILES, NT).transpose(2, 1, 0, 3)
        )
        shard8 = x8[0, :, t, :, :].reshape(C, PX)
        x8h = np.ascontiguousarray(
            shard8.reshape(KC, 128, NTILES, NT).transpose(2, 1, 0, 3)
        )
        m = {
            "x": xh,
            "x8": x8h,
            "w8": w8,
        }
        if has_bias:
            m["beff"] = np.ascontiguousarray(b_eff.reshape(KC, 128).T.astype(bf16))
        in_maps.append(m)
    return in_maps, has_bias


def _run(inputs: dict, **run_kwargs):
    in_maps, has_bias = _prep(**inputs)
    nc = _get_nc(has_bias)
    res = run_bass_kernel_spmd(nc, in_maps, core_ids=list(range(T)), **run_kwargs)
    b, c, t, h, w = 1, C, T, 64, 64
    out = np.empty((b, c, t, h, w), dtype=np.float32)
    for i in range(T):
        oh = res.results[i]["out"].astype(np.float32)  # [NTILES, 128, KC, NT]
        shard = oh.transpose(2, 1, 0, 3).reshape(c, PX)
        out[0, :, i, :, :] = shard.reshape(c, h, w)
    return out, res


def kernel(**inputs) -> np.ndarray:
    out, _ = _run(inputs)
    return out


# revision 28
# speedup vs baseline: 1.0040x; 1.0027x over previous
"""Trainium2 Bass kernel for nn_AttentionBlock (B=1, C=512, T=8, H=W=64).

Math: the reference's attention has seq-len 1 (softmax over a single
element == 1.0), so o == v and Q/K never affect the output:

    out = x + s(px) * (W_eff @ x)(px) + b_eff
    W_eff = w_proj @ w_v * gamma,  w_v = w_qkv[2C:3C]
    b_eff = w_proj @ b_v + b_proj
    s(px) = sqrt(C) / clip(||x[:, px]||, 1e-12)

(The per-pixel RMS scale s commutes through the channel contraction, so
the GEMM runs on raw x and s is applied to the GEMM output.)

Sharding: data-parallel over the fused (b*t)=8 frame axis, one frame per
NeuronCore; weights replicated. Per core the frame is shipped tile-major
([tile, p, chunk, n]: channels on partitions, pixels on the free dim).

Precision plan (rel-l2 budget is 2e-2; this lands ~8e-3):
 - residual identity + final store ride in bf16 (two roundings ~3e-3)
 - the GEMM runs in fp8e4m3 with DoubleRow perf mode (2 k-chunks per
   pass). W_eff is pre-scaled by 2^6 on the host — its entries (~1e-2)
   sit below e4m3's min normal otherwise — and the 2^-6 descale is
   folded into the Sqrt scale of the norm chain for free.
 - sumsq is computed from the fp8 copy of x (error ~2e-3 on s): the
   fp8 tile is a quarter the bytes and is shipped first, so the
   partition-reducing ones-matmuls never wait on the big bf16 tile and
   the warm PE never starves (a >3.4us PE gap would re-throttle the
   HAM clock gate to 1.2GHz).

Per-tile engine budget (8 tiles/core), balanced with on-HW measurements
(GPSIMD tensor ops run ~2.3x slower than DVE's; DVE all-bf16 ops get
2x mode; the psum-reading scale muls must be DVE):
  PE      8 DoubleRow mains + 4 sumsq-ones MMs    ~3.0us
  ACT     square (fp8->bf16) + sqrt               ~2.7us + store issue
  DVE     2 psum-scale muls + 1/4 residual add + reciprocal ~3.5us
  GPSIMD  3/4 residual add                        ~3.1us
Loads stream on the qSP HWDGE ring ([x8_t, x_t] per tile, weights after
x8_0); stores are issued per-tile on the qAct ring (one tile delayed so
a still-waiting store never head-of-line blocks the ACT compute queue)
and drain concurrently with the loads. A burst of throwaway matmuls on
constants bridges the preamble-to-first-data window so the PE's HAM
clock gate is already at 8/8 when the real stream begins.
"""

import numpy as np

import concourse.tile as tile
from concourse import bacc, mybir
from concourse.bass_utils import run_bass_kernel_spmd

C = 512  # channels
T = 8  # frames == cores
PX = 4096  # pixels per frame (64*64)
NT = 512  # pixel-tile (one PSUM bank of fp32)
NTILES = PX // NT  # 8
KC = C // 128  # 4 channel chunks

F32 = mybir.dt.float32
F32R = mybir.dt.float32r
BF16 = mybir.dt.bfloat16
FP8 = mybir.dt.float8e4

WSHIFT = 64.0  # w8 = W_eff.T * 2^6; descale folded into the Sqrt scale

# 1e-24/C: Sqrt((sumsq + 1e-24)/C) reproduces the reference's
# clip(norm, 1e-12) for all non-degenerate inputs. The norm chain
# computes WSHIFT*sqrt(.) so the reciprocal yields s/WSHIFT directly.
_EPS = 1e-24 / C

_BUILD_CACHE: dict = {}


def _build(has_bias: bool):
    """Trace + compile the per-core Tile program. Returns the Bacc."""
    nc = bacc.Bacc("TRN2", target_bir_lowering=False, debug=False, num_devices=T)

    # x/x8/out are tile-major on the host side ([tile, p, a, n]): each
    # pixel tile is one contiguous DRAM block with a single contiguous
    # line per partition. w8 is [p, a, j, m]: one contiguous 2KB row per
    # partition.
    x = nc.dram_tensor("x", [NTILES, 128, KC, NT], BF16, kind="ExternalInput").ap()
    x8 = nc.dram_tensor("x8", [NTILES, 128, KC, NT], FP8, kind="ExternalInput").ap()
    w8 = nc.dram_tensor("w8", [128, KC, KC, 128], FP8, kind="ExternalInput").ap()
    out = nc.dram_tensor("out", [NTILES, 128, KC, NT], BF16, kind="ExternalOutput").ap()
    beff = None
    if has_bias:
        beff = nc.dram_tensor("beff", [128, KC], BF16, kind="ExternalInput").ap()

    with tile.TileContext(nc) as tc:
        with (
            tc.tile_pool(name="const", bufs=1) as const,
            tc.tile_pool(name="xin", bufs=8) as xin,
            tc.tile_pool(name="x8in", bufs=8) as x8in,
            tc.tile_pool(name="sq", bufs=4) as sq,
            tc.tile_pool(name="sca", bufs=6) as sca,
            tc.tile_pool(name="tmp", bufs=4) as tmpp,
            tc.tile_pool(name="acc", bufs=3, space="PSUM") as accp,
            tc.tile_pool(name="stat", bufs=2, space="PSUM") as statp,
        ):
            ones_bf = const.tile([128, 128], F32)
            nc.vector.memset(ones_bf, 1.0)
            ones_b = const.tile([128, 128], BF16)
            nc.vector.tensor_copy(ones_b, ones_bf)
            eps_t = const.tile([128, 1], F32)
            nc.vector.memset(eps_t, _EPS * WSHIFT * WSHIFT)
            w8_sb = const.tile([128, KC, KC, 128], FP8)
            if has_bias:
                beff_sb = const.tile([128, KC], BF16)
                nc.sync.dma_start(out=beff_sb, in_=beff)

            # HAM warmup: the PE sits idle from the end of the framework
            # preamble (~7us) until the first operands land (~10.5us),
            # and the clock gate needs ~3.4us of sustained activity to
            # lift 1.2GHz -> 2.4GHz. Throwaway matmuls on constant tiles
            # bridge exactly that window so the real GEMM stream runs
            # warm from its first instruction.
            scr = const.tile([128, NT], BF16)
            nc.vector.memset(scr, 0.0)
            wacc = statp.tile([128, NT], F32, tag="stat", name="warm")
            for _ in range(8):
                nc.tensor.matmul(wacc, lhsT=ones_b, rhs=scr, start=True, stop=True)

            store_q = []  # (ti, xt) pairs whose store is still to be issued

            for ti in range(NTILES):
                x8t = x8in.tile([128, KC, NT], FP8, tag="x8t")
                nc.sync.dma_start(out=x8t, in_=x8[ti])
                if ti == 0:
                    nc.sync.dma_start(out=w8_sb, in_=w8)
                xt = xin.tile([128, KC, NT], BF16, tag="xt")
                nc.sync.dma_start(out=xt, in_=x[ti])

                # per-pixel sum of squares over channels, from the fp8
                # copy (x8 = quant(x), so x8^2 mis-estimates sumsq by
                # ~0.2% — far inside the error budget — and arrives a
                # full bf16-tile earlier): square on ACT, then four
                # ones[128,128] matmuls that reduce the partitions AND
                # broadcast the result (fp32 psum) to every partition.
                x2 = sq.tile([128, KC, NT], BF16, tag="x2", name="x2")
                nc.scalar.activation(
                    out=x2,
                    in_=x8t,
                    func=mybir.ActivationFunctionType.Square,
                )

                accs = []
                for jj in range(KC // 2):
                    accs.append(accp.tile([128, 2, NT], F32, tag="acc", name="acc"))
                ssb = statp.tile([128, NT], F32, tag="stat", name="ssb")

                # sumsq matmuls go FIRST on the PE queue (except tile 0,
                # where waiting for the first ACT square would delay the
                # very first GEMM): the scale chain (sqrt, recip) then
                # completes under the mains, so each combine fires the
                # moment its psum group stops and the psum ring buffer
                # (reused by tile t+1's second group) frees in time.
                def emit_ones():
                    for a in range(KC):
                        nc.tensor.matmul(
                            ssb,
                            lhsT=ones_b,
                            rhs=x2[:, a, :],
                            start=(a == 0),
                            stop=(a == KC - 1),
                        )

                def emit_mains(jjs):
                    # DoubleRow: each pass consumes a pair of ci-chunks
                    # (lhsT [128, 2, 128], rhs [128, 2, 512]), so a psum
                    # group accumulates in 2 passes instead of 4.
                    for jj in jjs:
                        for q in range(2):
                            for p in range(KC // 2):
                                nc.tensor.matmul(
                                    accs[jj][:, q, :],
                                    lhsT=w8_sb[:, 2 * p : 2 * p + 2, jj * 2 + q, :],
                                    rhs=x8t[:, 2 * p : 2 * p + 2, :],
                                    start=(p == 0),
                                    stop=(p == KC // 2 - 1),
                                    perf_mode=mybir.MatmulPerfMode.DoubleRow,
                                )

                if ti == 0:
                    # tile 0: sumsq slots between the two psum groups —
                    # its square input is ready by then (waiting for it
                    # up front would delay the very first GEMM), and the
                    # scale chain starts ~1.3us earlier than fully-after,
                    # which shrinks tile 1's second-group psum wait.
                    emit_mains([0])
                    emit_ones()
                    emit_mains([1])
                else:
                    emit_ones()
                    emit_mains(list(range(KC // 2)))

                # WSHIFT/sqrt-chain: stb = WSHIFT*sqrt(sumsq/C + eps),
                # so recip gives s/WSHIFT and the psum descale is free.
                stb = sca.tile([128, NT], F32R, tag="stb", name="stb")
                nc.scalar.activation(
                    out=stb,
                    in_=ssb,
                    func=mybir.ActivationFunctionType.Sqrt,
                    scale=WSHIFT * WSHIFT / C,
                    bias=eps_t,
                )
                sb_s = sca.tile([128, NT], F32, tag="sb_s", name="sb_s")
                nc.vector.reciprocal_approx_fast(out=sb_s, in_=stb.bitcast(F32))

                # combine: out = x + (s/WSHIFT)*acc (+beff). The
                # psum-reading muls must be DVE; the residual add is
                # split 3:1 GPSIMD:DVE to keep DVE at the PE's pace.
                sb_w = sb_s.unsqueeze(1).broadcast_to([128, 2, NT])
                tm = tmpp.tile([128, KC, NT], BF16, tag="tm", name="tm")
                nc.vector.tensor_mul(tm[:, 0:2, :], accs[0], sb_w)
                nc.vector.tensor_mul(tm[:, 2:4, :], accs[1], sb_w)
                if has_bias:
                    for j in range(KC):
                        nc.vector.tensor_add(
                            tm[:, j, :],
                            tm[:, j, :],
                            beff_sb[:, j : j + 1].broadcast_to([128, NT]),
                        )
                if ti < NTILES - 1:
                    nc.gpsimd.tensor_add(xt[:, 0:2, :], tm[:, 0:2, :], xt[:, 0:2, :])
                    nc.vector.tensor_add(xt[:, 2:4, :], tm[:, 2:4, :], xt[:, 2:4, :])
                else:
                    # the last tile's combine is on the critical path to
                    # the final store: run both halves on DVE (2x-mode
                    # bf16, ~3x faster than GPSIMD) and store each half
                    # as soon as its add lands.
                    nc.vector.tensor_add(xt[:, 0:2, :], tm[:, 0:2, :], xt[:, 0:2, :])
                    nc.vector.tensor_add(xt[:, 2:4, :], tm[:, 2:4, :], xt[:, 2:4, :])

                store_q.append((ti, xt))

            # stores ride the idle qSP ring, emitted after every load so
            # a waiting store can never block a load issue; the SP engine
            # just paces them behind each tile's combine. The last tile
            # stores in halves, each as soon as its add lands.
            for si, sxt in store_q[:-1]:
                nc.sync.dma_start(out=out[si], in_=sxt)
            (si, sxt) = store_q[-1]
            nc.sync.dma_start(out=out[si][:, 0:2, :], in_=sxt[:, 0:2, :])
            nc.sync.dma_start(out=out[si][:, 2:4, :], in_=sxt[:, 2:4, :])

    nc.compile()
    return nc


def _get_nc(has_bias: bool):
    key = has_bias
    if key not in _BUILD_CACHE:
        _BUILD_CACHE[key] = _build(has_bias)
    return _BUILD_CACHE[key]


def _prep(x, gamma, w_qkv, b_qkv, w_proj, b_proj):
    """Host-side shard + weight fold. Returns (in_maps, has_bias)."""
    bf16 = mybir.dt.np(BF16)
    fp8 = mybir.dt.np(FP8)
    x = np.asarray(x, dtype=np.float32)
    gamma = np.asarray(gamma, dtype=np.float32)
    w_qkv = np.asarray(w_qkv, dtype=np.float32)
    b_qkv = np.asarray(b_qkv, dtype=np.float32)
    w_proj = np.asarray(w_proj, dtype=np.float32)
    b_proj = np.asarray(b_proj, dtype=np.float32)

    w_v = w_qkv[2 * C : 3 * C, :]  # [cv, ci]
    b_v = b_qkv[2 * C : 3 * C]
    w_eff = (w_proj @ w_v) * gamma[None, :]  # [co, ci]
    # lhsT layout [p, a, j, m]: lhsT[a*128+p -> ci, j*128+m -> co].
    # Pre-scaled by WSHIFT so the ~1e-2 entries clear e4m3's min normal.
    w8 = np.ascontiguousarray(
        (w_eff.T * WSHIFT).reshape(KC, 128, KC, 128).transpose(1, 0, 2, 3).astype(fp8)
    )
    b_eff = (w_proj @ b_v + b_proj).astype(np.float32)
    has_bias = bool(np.any(b_eff != 0.0))

    xb = x.astype(bf16)
    x8 = x.astype(fp8)
    in_maps = []
    for t in range(T):
        shard = xb[0, :, t, :, :].reshape(C, PX)
        xh = np.ascontiguousarray(
            shard.reshape(KC, 128, NTILES, NT).transpose(2, 1, 0, 3)
        )
        shard8 = x8[0, :, t, :, :].reshape(C, PX)
        x8h = np.ascontiguousarray(
            shard8.reshape(KC, 128, NTILES, NT).transpose(2, 1, 0, 3)
        )
        m = {
            "x": xh,
            "x8": x8h,
            "w8": w8,
        }
        if has_bias:
            m["beff"] = np.ascontiguousarray(b_eff.reshape(KC, 128).T.astype(bf16))
        in_maps.append(m)
    return in_maps, has_bias


def _run(inputs: dict, **run_kwargs):
    in_maps, has_bias = _prep(**inputs)
    nc = _get_nc(has_bias)
    res = run_bass_kernel_spmd(nc, in_maps, core_ids=list(range(T)), **run_kwargs)
    b, c, t, h, w = 1, C, T, 64, 64
    out = np.empty((b, c, t, h, w), dtype=np.float32)
    for i in range(T):
        oh = res.results[i]["out"].astype(np.float32)  # [NTILES, 128, KC, NT]
        shard = oh.transpose(2, 1, 0, 3).reshape(c, PX)
        out[0, :, i, :, :] = shard.reshape(c, h, w)
    return out, res


def kernel(**inputs) -> np.ndarray:
    out, _ = _run(inputs)
    return out
